# revision 1
# baseline (speedup 1.0000x reference)
"""Trainium2 Bass kernel for nn_GCNNMultiKernel (gnn_message_passing).

Sharding: 8 cores = 4 graphs x 2 node-column halves. Core c owns graph
b = c // 2 and node columns [half*1024, half*1024+1024), half = c % 2.

Per-core device program (single graph b, C=1024 owned columns):
  - d2[m, n] for all m in [0,2048), owned n, built once in SBUF (8MB):
    PSUM = (-2*coord).T @ coord_my + ones.T @ r2row_my + r2col.T @ ones
    (bf16 operands, fp32 accum), then a plain DVE copy PSUM->SBUF.
  - 3 layers. Per (layer, 512-band, edge e):
      ACT: K tiles = Exp(scale_le * d2) in bf16, scale is a per-partition AP
      PE:  psum (64 x 512) += [embT | 0 | ones_F | 0].T @ K  (16 k-tiles)
           rows 0:fin    = adj_e = emb @ K_e
           rows 32:32+fin = deg_e replicated (via the ones columns)
      DVE: A_e copy PSUM->SBUF bf16; D_e = deg_rep * emb_loc (bf16)
    Stage 2 per band: psum(64x512) = sum over 9 ops wT_op.T @ spread_op +
    bias row outer-product; lin rows at 0:16, nlin rows at 32:48 (every
    PSUM/SBUF engine access starts at partition 0/32/64/96). ReLU'd nlin is
    staged at partition 32 and DMA-shifted into new_emb rows 16:32.
  - Between layers: pair AllGather of the un-normalized local emb via DRAM,
    spatialnorm stats from the gathered full emb (bn_stats/aggr, sqrt via
    Ln/Exp + one Newton step), normalize full + local, PE-transpose full
    emb into the next layer's lhsT.
  - After layer 3: per-core column-sum of final local emb -> output (32,).

Host: builds per-core inputs (slices, -2*coord, r2 layouts, stacked/
transposed weights in the padded row layout, -1/sigma^2 table) and computes
the tiny final head (mean-pool combine, InstanceNorm over 32 features, FC,
sigmoid) in numpy.

_build_program(repeat=K) wraps the whole compute body in a device For_i
loop; used only for timing (wall-clock difference between K and 1 reps).
"""
import sys
from contextlib import ExitStack

sys.path.insert(0, "/opt/trn_rl_repo")

import numpy as np
import ml_dtypes
import concourse.bass as bass
import concourse.bacc as bacc
import concourse.tile as tile
from concourse import mybir
from concourse.bass_utils import run_bass_kernel_spmd

F32 = mybir.dt.float32
F16 = mybir.dt.float16
BF16 = mybir.dt.bfloat16
AF = mybir.ActivationFunctionType
ALU = mybir.AluOpType

B = 4
N = 2048
C = 1024          # owned columns per core
P = 128           # SBUF partitions per m-tile
MT = N // P       # 16 m-tiles
NBW = 512         # band width (one PSUM bank of fp32)
NB = C // NBW     # 2 bands
E = 4             # edge kernels
L = 3             # layers
F0 = 16           # input feature dim (layer 0)
F = 32            # node feature dim
G = 16            # m-tiles per ACT exp instruction
KBUFS = 6         # kt tile ring
S1BUFS = 4        # stage-1 PSUM banks
ADBUFS = 2        # a_t/d_t rings
EPS_SN = 1e-8
EPS_IN = 1e-5

_CACHE = {}


def _build_program(repeat=1, fake_cc=False):
    nc = bacc.Bacc("TRN2", target_bir_lowering=False, debug=False, num_devices=8)

    # ------------- DRAM I/O (per-core data supplied via in_maps) -------------
    d_c2 = nc.dram_tensor("c2_all", (4, N), BF16, kind="ExternalInput")
    d_cm = nc.dram_tensor("coords_my", (4, C), BF16, kind="ExternalInput")
    d_sig = nc.dram_tensor("sigscale", (1, L * E), F32, kind="ExternalInput")
    d_embT0 = nc.dram_tensor("embT0_ones", (N, 2 * F), BF16, kind="ExternalInput")
    d_emb0 = nc.dram_tensor("emb0_loc", (F0, C), BF16, kind="ExternalInput")
    d_wT0 = nc.dram_tensor("wT0", (F0, 9, 2 * F), BF16, kind="ExternalInput")
    d_wT12 = nc.dram_tensor("wT12", (F, 2, 9, 2 * F), BF16, kind="ExternalInput")
    d_bcat = nc.dram_tensor("bcat", (1, L, 2 * F), BF16, kind="ExternalInput")
    d_ident = nc.dram_tensor("ident", (F, F), F32, kind="ExternalInput")

    d_pp = nc.dram_tensor("pooled_partial", (F, 1), F32, kind="ExternalOutput")

    # collective bounce buffers: one pair per (boundary, band) so each 512-band
    # gathers as soon as that band's layer output is ready
    cc_in = [[nc.dram_tensor(f"cc_in{i}_{j}", (F, NBW), F32) for j in range(NB)]
             for i in range(L - 1)]
    cc_out = [[nc.dram_tensor(f"cc_out{i}_{j}", (2, F, NBW), F32) for j in range(NB)]
              for i in range(L - 1)]
    groups = [[0, 1], [2, 3], [4, 5], [6, 7]]

    with tile.TileContext(nc) as tc, ExitStack() as ctx:
        consts = ctx.enter_context(tc.tile_pool(name="consts", bufs=1))
        d2pool = ctx.enter_context(tc.tile_pool(name="d2", bufs=1))
        kpool = ctx.enter_context(tc.tile_pool(name="ktile", bufs=KBUFS))
        adpool = ctx.enter_context(tc.tile_pool(name="spread", bufs=ADBUFS))
        embpool = ctx.enter_context(tc.tile_pool(name="emb", bufs=1))
        small = ctx.enter_context(tc.tile_pool(name="small", bufs=4))
        ps_misc = ctx.enter_context(tc.tile_pool(name="ps_misc", bufs=2, space="PSUM"))
        ps_s1 = ctx.enter_context(tc.tile_pool(name="ps_s1", bufs=S1BUFS, space="PSUM"))
        ps_s2 = ctx.enter_context(tc.tile_pool(name="ps_s2", bufs=2, space="PSUM"))

        # ------------------------- constants to SBUF -------------------------
        c2_sb = consts.tile([4, N], BF16)
        nc.sync.dma_start(out=c2_sb, in_=d_c2.ap())
        cm_sb = consts.tile([4, C], BF16)
        nc.sync.dma_start(out=cm_sb, in_=d_cm.ap())
        ones_n = consts.tile([1, NBW], BF16)
        nc.vector.memset(ones_n, 1.0)
        ident_sb = consts.tile([F, F], F32)
        nc.sync.dma_start(out=ident_sb, in_=d_ident.ap())
        bcat_sb = consts.tile([1, L, 2 * F], BF16)
        nc.sync.dma_start(out=bcat_sb, in_=d_bcat.ap())
        wT0_sb = consts.tile([F0, 9, 2 * F], BF16)
        nc.sync.dma_start(out=wT0_sb, in_=d_wT0.ap())
        wT12_sb = consts.tile([F, 2, 9, 2 * F], BF16)
        nc.sync.dma_start(out=wT12_sb, in_=d_wT12.ap())
        ones1 = consts.tile([1, P], BF16)
        nc.vector.memset(ones1, 1.0)
        # sigma scales broadcast across partitions: (128, 12)
        sig_sb = consts.tile([P, L * E], F32)
        sig_bcast = bass.AP(tensor=d_sig.ap().tensor, offset=0,
                            ap=[[0, P], [1, L * E]])
        nc.sync.dma_start(out=sig_sb, in_=sig_bcast)
        # layer-0 lhsT: [embT0 | 0 | ones16 | 0] rearranged (mt p) f -> p mt f
        embT0_sb = consts.tile([P, MT, 2 * F], BF16)
        nc.sync.dma_start(
            out=embT0_sb,
            in_=d_embT0.ap().rearrange("(mt p) f -> p mt f", p=P),
        )
        emb0_sb = consts.tile([F0, C], BF16)
        nc.sync.dma_start(out=emb0_sb, in_=d_emb0.ap())

        # lhsT for layers 1,2: per-mtile [embT | ones] tiles so next-layer
        # matmuls can start as soon as their own m-tile is transposed
        embT_tiles = []
        for _mt in range(MT):
            _t = embpool.tile([P, 2 * F], BF16, tag=f"embT{_mt}")
            nc.vector.memset(_t[:, F:2 * F], 1.0)
            embT_tiles.append(_t)

        # persistent emb tensors
        emb_loc = embpool.tile([F, C], BF16)       # normalized local (layers 1,2)
        new_emb = embpool.tile([F, C], F32)        # layer output (local cols)
        # gathered (un-normalized) emb quarters: q = rank*NB + nb;
        # quarter q holds node columns [q*NBW, (q+1)*NBW) in canonical order
        emb_q = []
        for _q in range(4):
            _t = embpool.tile([F, NBW], F32, tag=f"emb_q{_q}")
            emb_q.append(_t)

        warm = consts.tile([1, 1], F32)
        nc.vector.memset(warm, 0.0)
        nc.scalar.activation(warm, warm, AF.Exp, bias=0.0, scale=1.0)

        # stage-1 matmul issue order: quarters fed by the band-0 gather
        # (mt 0-3 and 8-11) first; PSUM accumulation is order-free
        TORDER = [0, 1, 2, 3, 8, 9, 10, 11, 4, 5, 6, 7, 12, 13, 14, 15]

        def _emit_body():
            # --------------------------- phase 1: d2 ---------------------------
            d2_sb = d2pool.tile([P, MT, C], F16)
            for nb in range(NB):
                for mt in range(MT):
                    ps = ps_misc.tile([P, NBW], F32, tag="misc")
                    nc.tensor.matmul(
                        ps,
                        c2_sb[:, mt * P:(mt + 1) * P],
                        cm_sb[:, nb * NBW:(nb + 1) * NBW],
                        start=True, stop=True)
                    nc.vector.tensor_copy(
                        d2_sb[:, mt, nb * NBW:(nb + 1) * NBW], ps)

            # -------------------------- phase 2: layers -------------------------
            for lay in range(L):
                fin = F0 if lay == 0 else F
                pass
                ecur = emb0_sb if lay == 0 else emb_loc
                pp_bands = None
                if lay == L - 1:
                    pp_bands = []
                    for _j in range(NB):
                        ppt = small.tile([F, 1], F32,
                                         tag=f"ppb{_j}", name=f"ppb{_j}")
                        pp_bands.append(ppt)
                for nb in range(NB):
                    ncols = slice(nb * NBW, (nb + 1) * NBW)
                    a_t = adpool.tile([F, E, NBW], BF16, tag="a_t")
                    d_t = adpool.tile([F, E, NBW], BF16, tag="d_t")
                    for e in range(E):
                        le = lay * E + e
                        ps1 = ps_s1.tile([2 * F, NBW], F32)
                        kts = []
                        for g in range(MT // G):
                            kt = kpool.tile([P, G, NBW], BF16, tag="kt")
                            if ((lay == 0 and nb == 0 and e == 0 and g == 0)
                                    or (lay == L - 1 and nb == NB - 1
                                        and e == E - 1 and g == MT // G - 1)):
                                # halve the very first exp op so ACT starts as
                                # soon as the first four d2 m-tiles exist
                                nc.scalar.activation(
                                    kt[:, 0:G // 2, :],
                                    d2_sb[:, g * G:g * G + G // 2, ncols],
                                    AF.Exp,
                                    bias=0.0, scale=sig_sb[:, le:le + 1])
                                nc.scalar.activation(
                                    kt[:, G // 2:G, :],
                                    d2_sb[:, g * G + G // 2:(g + 1) * G, ncols],
                                    AF.Exp,
                                    bias=0.0, scale=sig_sb[:, le:le + 1])
                            else:
                                nc.scalar.activation(
                                    kt, d2_sb[:, g * G:(g + 1) * G, ncols],
                                    AF.Exp,
                                    bias=0.0, scale=sig_sb[:, le:le + 1])
                            kts.append(kt)
                        # MMs in quarter-priority order: band-0 quarters first
                        for mi, mt in enumerate(TORDER):
                            lhsT_mt = (embT0_sb[:, mt, :] if lay == 0
                                       else embT_tiles[mt])
                            nc.tensor.matmul(
                                ps1,
                                lhsT_mt,
                                kts[mt // G][:, mt % G, :],
                                start=(mi == 0), stop=(mi == MT - 1))
                        if lay == 0:
                            nc.vector.tensor_copy(a_t[0:fin, e, :],
                                                  ps1[0:fin, :])
                        else:
                            # fold spatialnorm into the epilogue:
                            # A_norm = inv*A_un - (inv*mu)*deg_rep
                            dsc = small.tile([F, NBW], F32, tag="dsc")
                            nc.vector.tensor_scalar_mul(
                                dsc[0:fin, :], ps1[F:F + fin, :],
                                invmu[0:fin, 0:1])
                            nc.vector.scalar_tensor_tensor(
                                a_t[0:fin, e, :], ps1[0:fin, :],
                                inv[0:fin, 0:1], dsc[0:fin, :],
                                op0=ALU.mult, op1=ALU.subtract)
                        nc.vector.tensor_mul(
                            d_t[0:fin, e, :], ps1[F:F + fin, :], ecur[:, ncols])
                    # stage 2: 9 op-terms + bias accumulate into one PSUM bank
                    ps2 = ps_s2.tile([2 * F, NBW], F32)

                    def wt(op):
                        if lay == 0:
                            return wT0_sb[:, op, :]
                        return wT12_sb[:, lay - 1, op, :]

                    nc.tensor.matmul(ps2, wt(0), ecur[:, ncols],
                                     start=True, stop=False)
                    for e in range(E):
                        nc.tensor.matmul(ps2, wt(1 + e), d_t[0:fin, e, :],
                                         start=False, stop=False)
                    for e in range(E):
                        nc.tensor.matmul(ps2, wt(5 + e), a_t[0:fin, e, :],
                                         start=False, stop=False)
                    nc.tensor.matmul(ps2, bcat_sb[0:1, lay, :], ones_n,
                                     start=False, stop=True)
                    # epilogue: copy lin half; ReLU nlin half via partition-32
                    # scratch + DMA shift (engine APs start at 0/32/64/96)
                    nc.vector.tensor_copy(new_emb[0:F // 2, ncols],
                                          ps2[0:F // 2, :])
                    nlin_tmp = adpool.tile([F + F // 2, NBW], F32,
                                           tag="nlin_tmp")
                    nc.vector.tensor_scalar_max(nlin_tmp[F:F + F // 2, :],
                                                ps2[F:F + F // 2, :], 0.0)
                    nc.sync.dma_start(out=new_emb[F // 2:F, ncols],
                                      in_=nlin_tmp[F:F + F // 2, :])
                    if lay == L - 1:
                        ppn = pp_bands[nb]
                        nc.vector.reduce_sum(ppn, new_emb[:, ncols],
                                             axis=mybir.AxisListType.X)

                    if lay < L - 1:
                        # gather this band across the pair right away
                        nc.sync.dma_start(out=cc_in[lay][nb].ap(),
                                          in_=new_emb[:, ncols])
                        if fake_cc:
                            nc.sync.dma_start(out=cc_out[lay][nb][0],
                                              in_=cc_in[lay][nb].ap())
                            nc.sync.dma_start(out=cc_out[lay][nb][1],
                                              in_=cc_in[lay][nb].ap())
                        else:
                            nc.gpsimd.collective_compute(
                                "AllGather", ALU.bypass, replica_groups=groups,
                                ins=[cc_in[lay][nb].ap().opt()],
                                outs=[cc_out[lay][nb].ap().opt()])
                        nc.sync.dma_start(out=emb_q[nb],
                                          in_=cc_out[lay][nb][0])
                        nc.sync.dma_start(out=emb_q[NB + nb],
                                          in_=cc_out[lay][nb][1])

                if lay < L - 1:
                    # transposes straight off the un-normalized quarters
                    # (mtile mt lives in quarter mt // (MT // 4))
                    for mt in TORDER:
                        q = mt // (MT // 4)
                        lo = (mt % (MT // 4)) * P
                        pst_full = ps_misc.tile([P, NBW], F32, tag="misc")
                        pst = pst_full[:, 0:F]
                        nc.tensor.transpose(
                            pst, emb_q[q][:, lo:lo + P], ident_sb)
                        nc.vector.tensor_copy(embT_tiles[mt][:, 0:F], pst)

                    # spatialnorm stats over full N per feature row
                    stats = small.tile([F, 4, 6], F32)
                    for sg in range(4):
                        nc.vector.bn_stats(out=stats[:, sg, :],
                                           in_=emb_q[sg])
                    mv = small.tile([F, 2], F32)
                    nc.vector.bn_aggr(out=mv, in_=stats)
                    varu = small.tile([F, 1], F32)
                    nc.vector.tensor_scalar_mul(varu, mv[:, 1:2],
                                                float(N) / (N - 1))
                    # rsqrt on DVE only (keeps ACT on the exp table set):
                    # bit-trick seed + 3 Newton iterations, then sd = v*y
                    yr = small.tile([F, 1], F32)
                    iv = varu.bitcast(mybir.dt.int32)
                    nc.vector.tensor_scalar(yr.bitcast(mybir.dt.int32), iv,
                                            1, None, op0=ALU.logical_shift_right)
                    nc.vector.tensor_scalar(yr.bitcast(mybir.dt.int32),
                                            yr.bitcast(mybir.dt.int32),
                                            0xFFFFFFFF, None,
                                            op0=ALU.bitwise_xor)
                    nc.vector.tensor_scalar(yr.bitcast(mybir.dt.int32),
                                            yr.bitcast(mybir.dt.int32),
                                            0x5f3759df + 1, None,
                                            op0=ALU.add)
                    t_a = small.tile([F, 1], F32)
                    for _ in range(3):
                        nc.vector.tensor_mul(t_a, yr, yr)
                        nc.vector.tensor_mul(t_a, t_a, varu)
                        nc.vector.tensor_scalar(t_a, t_a, -0.5, 1.5,
                                                op0=ALU.mult, op1=ALU.add)
                        nc.vector.tensor_mul(yr, yr, t_a)
                    sd = small.tile([F, 1], F32)
                    nc.vector.tensor_mul(sd, varu, yr)
                    den = small.tile([F, 1], F32)
                    nc.vector.tensor_scalar_add(den, sd, EPS_SN)
                    inv = small.tile([F, 1], F32, tag="inv")
                    nc.vector.reciprocal(inv, den)
                    invmu = small.tile([F, 1], F32, tag="invmu")
                    nc.vector.tensor_mul(invmu, inv, mv[:, 0:1])
                    # normalized local emb for stage 2 of the next layer
                    nc.vector.tensor_scalar(emb_loc, new_emb, mv[:, 0:1],
                                            inv[:, 0:1],
                                            op0=ALU.subtract, op1=ALU.mult)
                else:
                    pp = small.tile([F, 1], F32)
                    nc.vector.tensor_add(pp, pp_bands[0], pp_bands[1])
                    nc.sync.dma_start(out=d_pp.ap(), in_=pp)

        if repeat > 1:
            with tc.For_i(0, repeat, 1):
                _emit_body()
        else:
            _emit_body()

    nc.compile()
    return nc


def _host_inputs(global_input, sigmas, w_lin0, b_lin0, w_nlin0, b_nlin0,
                 w_lin, b_lin, w_nlin, b_nlin):
    gi = np.asarray(global_input, np.float32)
    sig = np.asarray(sigmas, np.float32)
    # stage-2 output rows live at 0:16 (lin) and 32:48 (nlin) so every PSUM
    # slice starts at partition 0 or 32
    wl0 = np.asarray(w_lin0, np.float32)
    wn0 = np.asarray(w_nlin0, np.float32)
    wT0 = np.zeros((F0, 9, 2 * F), np.float32)
    for op in range(9):
        wT0[:, op, 0:16] = wl0[:, op * F0:(op + 1) * F0].T
        wT0[:, op, F:F + 16] = wn0[:, op * F0:(op + 1) * F0].T
    wT12 = np.zeros((F, 2, 9, 2 * F), np.float32)
    for l in range(2):
        wl = np.asarray(w_lin[l], np.float32)
        wn = np.asarray(w_nlin[l], np.float32)
        for op in range(9):
            wT12[:, l, op, 0:16] = wl[:, op * F:(op + 1) * F].T
            wT12[:, l, op, F:F + 16] = wn[:, op * F:(op + 1) * F].T
    bcat = np.zeros((1, L, 2 * F), np.float32)
    bl = [np.asarray(b_lin0, np.float32), np.asarray(b_lin[0], np.float32),
          np.asarray(b_lin[1], np.float32)]
    bn = [np.asarray(b_nlin0, np.float32), np.asarray(b_nlin[0], np.float32),
          np.asarray(b_nlin[1], np.float32)]
    for l in range(L):
        bcat[0, l, 0:16] = bl[l]
        bcat[0, l, F:F + 16] = bn[l]
    sigscale = (-1.0 / (sig.reshape(-1) ** 2)).reshape(1, L * E).astype(np.float32)
    ident = np.eye(F, dtype=np.float32)

    in_maps = []
    for c in range(8):
        b = c // 2
        half = c % 2
        cols = slice(half * C, half * C + C)
        coord = gi[b, :2, :]                      # (2, 2048)
        r2 = (coord ** 2).sum(axis=0)             # (2048,)
        c2r2 = np.empty((4, N), np.float32)       # lhsT rows [-2x, -2y, 1, r2]
        c2r2[0:2] = -2.0 * coord
        c2r2[2] = 1.0
        c2r2[3] = r2
        crm = np.empty((4, C), np.float32)        # rhs rows [x, y, r2, 1]
        crm[0:2] = coord[:, cols]
        crm[2] = r2[cols]
        crm[3] = 1.0
        embT0 = np.zeros((N, 2 * F), np.float32)
        embT0[:, 0:F0] = gi[b].T
        embT0[:, F:F + F0] = 1.0
        in_maps.append(dict(
            c2_all=c2r2.astype(ml_dtypes.bfloat16),
            coords_my=crm.astype(ml_dtypes.bfloat16),
            sigscale=sigscale,
            embT0_ones=embT0.astype(ml_dtypes.bfloat16),
            emb0_loc=np.ascontiguousarray(gi[b][:, cols]).astype(ml_dtypes.bfloat16),
            wT0=wT0.astype(ml_dtypes.bfloat16),
            wT12=wT12.astype(ml_dtypes.bfloat16),
            bcat=np.ascontiguousarray(bcat).astype(ml_dtypes.bfloat16),
            ident=ident,
        ))
    return in_maps


def kernel(global_input, sigmas, w_lin0, b_lin0, w_nlin0, b_nlin0,
           w_lin, b_lin, w_nlin, b_nlin, fcl_w, fcl_b):
    if "nc" not in _CACHE:
        _CACHE["nc"] = _build_program()
    nc = _CACHE["nc"]
    in_maps = _host_inputs(global_input, sigmas, w_lin0, b_lin0, w_nlin0,
                           b_nlin0, w_lin, b_lin, w_nlin, b_nlin)
    res = run_bass_kernel_spmd(nc, in_maps, core_ids=list(range(8)))
    pooled = np.empty((B, F), np.float64)
    for b in range(B):
        s0 = res.results[2 * b]["pooled_partial"].reshape(F)
        s1 = res.results[2 * b + 1]["pooled_partial"].reshape(F)
        pooled[b] = (s0.astype(np.float64) + s1.astype(np.float64)) / N
    mu = pooled.mean(axis=1, keepdims=True)
    var = pooled.var(axis=1, keepdims=True)
    normed = (pooled - mu) / np.sqrt(var + EPS_IN)
    logits = normed @ np.asarray(fcl_w, np.float64).T + np.asarray(fcl_b, np.float64)
    out = 1.0 / (1.0 + np.exp(-logits[:, 0]))
    return out.astype(np.float32)

